# revision 37
# baseline (speedup 1.0000x reference)
"""Trainium2 Bass kernel for packed varlen multi-head attention (AudioEncoderAttention).

Contract: kernel(**inputs) takes the FULL unsharded inputs (hidden_states
[8192,1024] packed as 8 sequences of 1024 tokens) and returns the FULL output
[8192,1024]. The 8 sequences are sharded one-per-NeuronCore (sequence
parallel); every core runs the same single-core program on its own sequence.

Single fused pipeline per core (T=1024 tokens, E=1024, H=16 heads, D=64),
structured to keep the PE (tensor) engine - the bottleneck at ~215us of
matmul work - continuously busy:

  v = x Wv^T first (interleaved with the j=0 q/k projections), then one
  software-pipelined loop over head pairs j: S^T = k^T.T q^T scores for pair
  j, exp on ACT straight out of PSUM, U = v~^T expS accumulation (with an
  appended ones-column producing softmax denominators as U row 64) in two
  free-dim half passes, while the j+1 q/k projections run on spare PE slots.
  Per-pair normalization (DVE reciprocal + DRAM-bounce broadcast DMA + one
  multiply; PE outer-product broadcast for the last pair to shorten the
  tail) is pipelined under the following matmuls. Finally
  y = attn^T.T woT + bo.

RoPE uses an interleaved head-dim layout (host permutes wq/wk output
channels so rotate-half pairs (i, i+32) sit in adjacent partitions): the
rotation becomes a single DVE stream_shuffle (even/odd partition swap per
32-quadrant) plus two fused scalar_tensor_tensor multiplies (which also
carry the q bias and its rotated pair) and an add - no DMA shuffles and no
PSUM->SBUF staging copy. The v bias is absorbed into bo via softmax rows
summing to 1, and bo is added on the DVE during the PSUM->SBUF output copy
against a partition-broadcast bias tile.

All matmuls run bf16 operands with fp32 PSUM accumulation; RoPE and softmax
stay fp32. Output is stored bf16 and widened to f32 on the host.
"""

import numpy as np
import ml_dtypes

import concourse.mybir as mybir
import concourse.tile as tile
from concourse import bacc
from concourse.bass_utils import run_bass_kernel_spmd

F32 = mybir.dt.float32
BF16 = mybir.dt.bfloat16
AF = mybir.ActivationFunctionType
MUL = mybir.AluOpType.mult
ADD = mybir.AluOpType.add
BF = ml_dtypes.bfloat16

NCORES = 8
T = 1024          # tokens per sequence (= per core)
E = 1024          # embed dim
H = 16            # heads
D = 64            # head dim
P = 128
NE = E // P       # e-chunks (contraction)
NI = E // P       # i-chunks (qkv output channels; head pair j = chunk j)
NT = T // P       # t-chunks
HALF = 512        # one PSUM bank of f32

SWAP_MASK = []
for _m in range(16):
    SWAP_MASK += [2 * _m + 1, 2 * _m]


def build_nc(loop_n=1, y_accum=False):
    nc = bacc.Bacc("TRN2", target_bir_lowering=False, debug=False)

    xT_d = nc.dram_tensor("xT", [P, NE, T], BF16, kind="ExternalInput").ap()
    wq_d = nc.dram_tensor("wq", [P, NI, NE, P], BF16, kind="ExternalInput").ap()
    wk_d = nc.dram_tensor("wk", [P, NI, NE, P], BF16, kind="ExternalInput").ap()
    wv_d = nc.dram_tensor("wv", [P, 2, NE, HALF], BF16, kind="ExternalInput").ap()
    wo_d = nc.dram_tensor("wo", [P, NI, E], BF16, kind="ExternalInput").ap()
    bqc_d = nc.dram_tensor("bqc", [P, NI], F32, kind="ExternalInput").ap()
    bqr_d = nc.dram_tensor("bqr", [P, NI], F32, kind="ExternalInput").ap()
    bor_d = nc.dram_tensor("bor", [1, E], F32, kind="ExternalInput").ap()
    cos_d = nc.dram_tensor("cosT", [P, T], F32, kind="ExternalInput").ap()
    sin_d = nc.dram_tensor("sinS", [P, T], F32, kind="ExternalInput").ap()
    y_d = nc.dram_tensor("y", [T, E], BF16, kind="ExternalOutput").ap()
    rscr = nc.dram_tensor("rscr", [NI, 2, T], F32, kind="Internal").ap()

    with tile.TileContext(nc) as tc:
        with tc.tile_pool(name="const", bufs=1) as cpool, \
             tc.tile_pool(name="main", bufs=1) as mpool, \
             tc.tile_pool(name="work", bufs=1) as wpool, \
             tc.tile_pool(name="psS", bufs=2, space="PSUM") as spool, \
             tc.tile_pool(name="psB", bufs=2, space="PSUM") as bank, \
             tc.tile_pool(name="psU", bufs=2, space="PSUM") as upool:

            # ---- constants / weights ------------------------------------
            # V's inputs arrive first: xT halves on the two HWDGE queues in
            # parallel, wv halves next on separate queues.
            ones64 = cpool.tile([1, HALF], BF16, tag="ones64")
            nc.gpsimd.memset(ones64, 1.0)
            x0 = mpool.tile([P, 2, T], BF16, tag="x0")
            nc.sync.dma_start(out=x0, in_=xT_d[:, 0:2, :])
            xb = mpool.tile([P, 4, T], BF16, tag="xb")
            nc.scalar.dma_start(out=xb, in_=xT_d[:, 4:8, :])
            x1 = mpool.tile([P, 2, T], BF16, tag="x1")
            nc.sync.dma_start(out=x1, in_=xT_d[:, 2:4, :])

            def x_ec(ec):
                if ec < 2:
                    return x0[:, ec, :]
                if ec < 4:
                    return x1[:, ec - 2, :]
                return xb[:, ec - 4, :]

            wvl = mpool.tile([P, NE, HALF], BF16, tag="wvl")
            nc.gpsimd.dma_start(out=wvl[:, 0:2], in_=wv_d[:, 0, 0:2])
            nc.gpsimd.dma_start(out=wvl[:, 2:4], in_=wv_d[:, 0, 2:4])
            nc.gpsimd.dma_start(out=wvl[:, 4:8], in_=wv_d[:, 0, 4:8])
            wvh = mpool.tile([P, NE, HALF], BF16, tag="wvh")
            nc.scalar.dma_start(out=wvh, in_=wv_d[:, 1])
            wq_t = mpool.tile([P, NI, NE, P], BF16, tag="wq")
            nc.sync.dma_start(out=wq_t, in_=wq_d)
            wk_t = mpool.tile([P, NI, NE, P], BF16, tag="wk")
            nc.scalar.dma_start(out=wk_t, in_=wk_d)

            cos_sb = cpool.tile([P, T], F32, tag="cos")
            nc.gpsimd.dma_start(out=cos_sb, in_=cos_d)
            sin_sb = cpool.tile([P, T], F32, tag="sin")
            nc.gpsimd.dma_start(out=sin_sb, in_=sin_d)
            wo_t = mpool.tile([P, NI, E], BF16, tag="wo")
            nc.gpsimd.dma_start(out=wo_t, in_=wo_d)
            bq_sb = cpool.tile([P, NI], F32, tag="bq")
            nc.sync.dma_start(out=bq_sb, in_=bqc_d)
            bqr_sb = cpool.tile([P, NI], F32, tag="bqr")
            nc.sync.dma_start(out=bqr_sb, in_=bqr_d)
            vt = mpool.tile([P, NT, H, D + 1], BF16, tag="vt")
            nc.gpsimd.memset(vt[:, :, :, D:D + 1], 1.0)
            attnT = mpool.tile([P, NI, T], BF16, tag="attnT")

            # PE warm-up: dummy outer-products ramp the tensor-engine
            # clock while the real inputs stream in. The scratch results
            # land in bo_b, which the real bo broadcast DMA then overwrites.
            bo_b = cpool.tile([P, E], F32, tag="bo")
            for jh in range(2):
                sl = slice(jh * HALF, (jh + 1) * HALF)
                psw = bank.tile([P, HALF], F32, tag="bank", name=f"warm{jh}")
                for r in range(3):
                    nc.tensor.matmul(psw, ones64[:, 0:P], ones64,
                                     start=True, stop=True)
                nc.vector.tensor_copy(out=bo_b[:, sl], in_=psw)
            nc.gpsimd.dma_start(out=bo_b, in_=bor_d.to_broadcast([P, E]))

            # ---- helpers ------------------------------------------------
            class Proj:
                """q or k projection for chunk ic; emitted as two PE groups.

                RoPE runs per token-half straight out of PSUM on the DVE:
                stream_shuffle for rotate-half (interleaved layout), then
                scalar_tensor_tensor fusing the (q) bias add with the
                cos/sin multiplies - no PSUM->SBUF copy at all."""

                def __init__(self, which, ic):
                    self.w_t = wq_t if which == "q" else wk_t
                    self.biased = which == "q"
                    self.ic = ic
                    self.name = f"{which}{ic}"
                    self.raw = wpool.tile([P, T], BF16, tag="raw", bufs=2,
                                          name=f"raw_{self.name}")
                    # stream_shuffle cannot convert dtypes (walrus ISA check):
                    # shuf must stay f32 to match the PSUM input
                    self.shuf = wpool.tile([P, T], F32, tag="shuf", bufs=2,
                                           name=f"shuf_{self.name}")
                    self.dst = wpool.tile([P, T], BF16, tag=which + "j", bufs=2,
                                          name=f"dst_{self.name}")

                def th(self, th):
                    ps = bank.tile([P, HALF], F32, tag="bank",
                                   name=f"ps_{self.name}{th}")
                    sl = slice(th * HALF, (th + 1) * HALF)
                    for ec in range(NE):
                        nc.tensor.matmul(ps, self.w_t[:, self.ic, ec, :],
                                         x_ec(ec)[:, sl],
                                         start=(ec == 0), stop=(ec == NE - 1))
                    ic = self.ic
                    b = bq_sb[:, ic:ic + 1] if self.biased else 0.0
                    br = bqr_sb[:, ic:ic + 1] if self.biased else 0.0
                    nc.vector.stream_shuffle(self.shuf[:, sl], ps, SWAP_MASK)
                    nc.vector.scalar_tensor_tensor(
                        out=self.raw[:, sl], in0=ps, scalar=b,
                        in1=cos_sb[:, sl], op0=ADD, op1=MUL)
                    nc.vector.scalar_tensor_tensor(
                        out=self.shuf[:, sl], in0=self.shuf[:, sl], scalar=br,
                        in1=sin_sb[:, sl], op0=ADD, op1=MUL)
                    nc.vector.tensor_tensor(out=self.dst[:, sl],
                                            in0=self.raw[:, sl],
                                            in1=self.shuf[:, sl], op=ADD)

            def v_half(tcb, ih):
                tb = slice(tcb * P, (tcb + 1) * P)
                wvt = wvl if ih == 0 else wvh
                psv = bank.tile([P, HALF], F32, tag="bank",
                                name=f"psv_{tcb}_{ih}")
                for ec in range(NE):
                    nc.tensor.matmul(psv, x_ec(ec)[:, tb], wvt[:, ec, :],
                                     start=(ec == 0), stop=(ec == NE - 1))
                nc.vector.tensor_copy(
                    out=vt[:, tcb, ih * 8:(ih + 1) * 8, 0:D],
                    in_=psv.rearrange("p (h d) -> p h d", d=D))

            # ---- phase 0: v projection + q0/k0, interleaved -------------
            q_cur = Proj("q", 0)
            k_cur = Proj("k", 0)
            # ih1 lags ih0 by three t-blocks so wvh's DMA stays ahead of use
            v_half(0, 0)
            v_half(1, 0)
            v_half(2, 0)
            v_half(0, 1)
            q_cur.th(0)
            v_half(3, 0)
            v_half(1, 1)
            q_cur.th(1)
            v_half(4, 0)
            v_half(2, 1)
            k_cur.th(0)
            v_half(5, 0)
            v_half(3, 1)
            k_cur.th(1)
            v_half(6, 0)
            v_half(4, 1)
            v_half(7, 0)
            v_half(5, 1)
            v_half(6, 1)
            v_half(7, 1)

            # ---- attention loop over head pairs -------------------------
            def s_unit(j, tcb, qT, kT, expS):
                """scores + exp for both heads of pair j at t-block tcb."""
                tb = slice(tcb * P, (tcb + 1) * P)
                for ph in range(2):
                    pb = ph * 64
                    pss = spool.tile([P, T], F32, tag="S",
                                     name=f"pss_{j}_{tcb}_{ph}")
                    for lc in range(2):
                        sl = slice(lc * HALF, (lc + 1) * HALF)
                        nc.tensor.matmul(pss[:, sl], kT[pb:pb + 64, tb],
                                         qT[pb:pb + 64, sl],
                                         start=True, stop=True)
                    es = wpool.tile([P, T], BF16, tag="expS", bufs=22,
                                    name=f"es_{j}_{tcb}_{ph}")
                    expS[(tcb, ph)] = es
                    nc.scalar.activation(out=es, in_=pss, func=AF.Exp)

            def u_unit(j, tcb, lc, psu, expS):
                sl = slice(lc * HALF, (lc + 1) * HALF)
                for ph in range(2):
                    nc.tensor.matmul(psu[ph], vt[:, tcb, 2 * j + ph, :],
                                     expS[(tcb, ph)][:, sl], start=(tcb == 0),
                                     stop=(tcb == NT - 1))

            deferred = []

            def normalize_half(j, lc, psu, defer=False):
                """attnT[:, j, half] = psu rows / psu row 64, pipelined.

                For the last pair the partition-broadcast runs as a PE
                outer-product (no DMA on the tail); with defer=True those PE
                matmuls + the final multiply are emitted later via
                finish_deferred() so the reciprocal latency hides behind
                other PE work."""
                sl = slice(lc * HALF, (lc + 1) * HALF)
                last = j == NI - 1
                if not last:
                    rb = wpool.tile([P, HALF], F32, tag="rb", bufs=2,
                                    name=f"rb_{j}_{lc}")
                rcs = []
                for ph in range(2):
                    # lc1 copies ride the DVE mid-loop (ACT is exp-bound
                    # there) but ACT for the last pair, whose exps are done
                    # - parallel with the DVE reciprocals on the tail.
                    if lc == 0 or last:
                        nc.scalar.activation(
                            out=attnT[ph * 64:(ph + 1) * 64, j, sl],
                            in_=psu[ph][0:D, :], func=AF.Copy)
                    else:
                        nc.vector.tensor_copy(
                            out=attnT[ph * 64:(ph + 1) * 64, j, sl],
                            in_=psu[ph][0:D, :])
                    rc = wpool.tile([1, HALF], BF16 if last else F32,
                                    tag=f"rc{ph}", bufs=2,
                                    name=f"rc_{j}_{ph}_{lc}")
                    if last:
                        with nc.allow_low_precision(
                                reason="bf16 recip feeds PE broadcast; "
                                "0.4% on 2 heads is inside tolerance"):
                            nc.vector.reciprocal(out=rc, in_=psu[ph][D:D + 1, :])
                        rcs.append(rc)
                    else:
                        nc.vector.reciprocal(out=rc, in_=psu[ph][D:D + 1, :])
                        eng = nc.sync if ph == 0 else nc.gpsimd
                        eng.dma_start(out=rscr[j, ph:ph + 1, sl], in_=rc)
                        eng.dma_start(
                            out=rb[ph * 64:(ph + 1) * 64, :],
                            in_=rscr[j, ph:ph + 1, sl].to_broadcast([64, HALF]))
                if last:
                    def fin(j=j, sl=sl, rcs=rcs, lc=lc):
                        rbp = spool.tile([P, T], F32, tag="S",
                                         name=f"rbp_{j}_{lc}")[:, 0:HALF]
                        for ph in range(2):
                            nc.tensor.matmul(rbp[ph * 64:(ph + 1) * 64, :],
                                             ones64[:, 0:D], rcs[ph],
                                             start=True, stop=True)
                        nc.vector.tensor_tensor(out=attnT[:, j, sl],
                                                in0=attnT[:, j, sl],
                                                in1=rbp, op=MUL)
                    if defer:
                        deferred.append(fin)
                    else:
                        fin()
                else:
                    nc.vector.tensor_tensor(out=attnT[:, j, sl],
                                            in0=attnT[:, j, sl],
                                            in1=rb, op=MUL)

            def finish_deferred():
                while deferred:
                    deferred.pop(0)()

            # Y accumulation groups opened early (icK 0..6 need only
            # already-normalized attnT chunks); icK=7 lands after the last
            # pair's normalization.
            pre_psy = {}

            def y_start(tcb, jh):
                tb = slice(tcb * P, (tcb + 1) * P)
                sl = slice(jh * HALF, (jh + 1) * HALF)
                psy = bank.tile([P, HALF], F32, tag="bank",
                                name=f"psy_{tcb}_{jh}")
                for icK in range(NI - 1):
                    nc.tensor.matmul(psy, attnT[:, icK, tb], wo_t[:, icK, sl],
                                     start=(icK == 0), stop=False)
                pre_psy[(tcb, jh)] = psy

            def y_full(tcb):
                tb = slice(tcb * P, (tcb + 1) * P)
                yst = wpool.tile([P, E], BF16, tag="yst", bufs=2,
                                 name=f"yst_{tcb}")
                for jh in range(2):
                    sl = slice(jh * HALF, (jh + 1) * HALF)
                    if (tcb, jh) in pre_psy:
                        psy = pre_psy.pop((tcb, jh))
                        nc.tensor.matmul(psy, attnT[:, NI - 1, tb],
                                         wo_t[:, NI - 1, sl],
                                         start=False, stop=True)
                    else:
                        psy = bank.tile([P, HALF], F32, tag="bank",
                                        name=f"psy_{tcb}_{jh}")
                        for icK in range(NI):
                            nc.tensor.matmul(psy, attnT[:, icK, tb],
                                             wo_t[:, icK, sl],
                                             start=(icK == 0),
                                             stop=(icK == NI - 1))
                    if tcb == NT - 1:
                        # split the final store for a shorter drain tail
                        for q in range(2):
                            qs = slice(jh * HALF + q * 256,
                                       jh * HALF + (q + 1) * 256)
                            nc.vector.tensor_tensor(out=yst[:, qs],
                                                    in0=psy[:, q * 256:
                                                            (q + 1) * 256],
                                                    in1=bo_b[:, qs], op=ADD)
                            eng = (nc.sync, nc.scalar)[q]
                            eng.dma_start(out=y_d[tb, qs], in_=yst[:, qs])
                    else:
                        nc.vector.tensor_tensor(out=yst[:, sl], in0=psy,
                                                in1=bo_b[:, sl], op=ADD)
                        eng = (nc.sync, nc.scalar)[jh]
                        eng.dma_start(out=y_d[tb, sl], in_=yst[:, sl])

            # The first five S units of pair j are emitted during pair
            # j-1's second U half-pass (phase 0 for j=0), keeping the ACT
            # exp stream fed across pair boundaries.
            expS_nxt = {}
            for t in range(5):
                s_unit(0, t, q_cur.dst, k_cur.dst, expS_nxt)

            for j in range(NI):
                expS = expS_nxt
                expS_nxt = {}
                qT, kT = q_cur.dst, k_cur.dst
                nxt = None
                if j + 1 < NI:
                    nxt = (Proj("q", j + 1), Proj("k", j + 1))
                psu = [[upool.tile([D + 1, HALF], F32, tag="U",
                                   name=f"U_{j}_{ph}_{lc}")
                        for ph in range(2)] for lc in range(2)]

                if nxt:
                    nxt[0].th(0)
                else:
                    y_start(0, 0)
                u_unit(j, 0, 0, psu[0], expS)
                s_unit(j, 5, qT, kT, expS)
                if nxt:
                    nxt[0].th(1)
                else:
                    y_start(0, 1)
                u_unit(j, 1, 0, psu[0], expS)
                s_unit(j, 6, qT, kT, expS)
                if nxt:
                    nxt[1].th(0)
                u_unit(j, 2, 0, psu[0], expS)
                s_unit(j, 7, qT, kT, expS)
                if nxt:
                    nxt[1].th(1)
                u_unit(j, 3, 0, psu[0], expS)
                u_unit(j, 4, 0, psu[0], expS)
                u_unit(j, 5, 0, psu[0], expS)
                u_unit(j, 6, 0, psu[0], expS)
                u_unit(j, 7, 0, psu[0], expS)
                if nxt:
                    normalize_half(j, 0, psu[0])
                    s_unit(j + 1, 0, nxt[0].dst, nxt[1].dst, expS_nxt)
                else:
                    normalize_half(j, 0, psu[0], defer=True)
                u_unit(j, 0, 1, psu[1], expS)
                u_unit(j, 1, 1, psu[1], expS)
                if not nxt:
                    finish_deferred()
                    y_full(0)
                if nxt:
                    s_unit(j + 1, 1, nxt[0].dst, nxt[1].dst, expS_nxt)
                u_unit(j, 2, 1, psu[1], expS)
                if nxt:
                    s_unit(j + 1, 2, nxt[0].dst, nxt[1].dst, expS_nxt)
                else:
                    y_full(1)
                u_unit(j, 3, 1, psu[1], expS)
                u_unit(j, 4, 1, psu[1], expS)
                if nxt:
                    s_unit(j + 1, 3, nxt[0].dst, nxt[1].dst, expS_nxt)
                else:
                    y_full(2)
                u_unit(j, 5, 1, psu[1], expS)
                u_unit(j, 6, 1, psu[1], expS)
                if nxt:
                    s_unit(j + 1, 4, nxt[0].dst, nxt[1].dst, expS_nxt)
                u_unit(j, 7, 1, psu[1], expS)
                normalize_half(j, 1, psu[1])
                if nxt:
                    q_cur, k_cur = nxt

            # ---- output projection --------------------------------------
            for tcb in (3, 4, 5, 6, 7):
                y_full(tcb)

    nc.compile()
    return nc


def _rope_tables(cos_s, sin_s):
    """Interleaved-layout [P, T] cos / signed-sin tables (f32)."""
    c64 = np.ascontiguousarray(cos_s.T.astype(np.float32))   # [64, T]
    s64 = np.ascontiguousarray(sin_s.T.astype(np.float32))
    idx = np.repeat(np.arange(32), 2)                        # freq per d' in 0..63
    sign = np.where(np.arange(64) % 2 == 0, -1.0, 1.0).astype(np.float32)
    cos_half = c64[idx]                                      # [64, T]
    sin_half = s64[idx] * sign[:, None]
    return (np.concatenate([cos_half, cos_half], axis=0),
            np.concatenate([sin_half, sin_half], axis=0))


def prep_core_inputs(x_s, cos_s, sin_s, shared):
    """Per-core input dict: x_s [1024, 1024] f32, cos_s/sin_s [1024, 64]."""
    d = dict(shared)
    d["xT"] = np.ascontiguousarray(
        x_s.T.reshape(NE, P, T).transpose(1, 0, 2)).astype(BF)
    d["cosT"], d["sinS"] = _rope_tables(cos_s, sin_s)
    return d


def _perm():
    """Interleave rotate-half pairs: per head, new channel 2i <- i, 2i+1 <- i+32."""
    p = []
    for h in range(H):
        for i in range(32):
            p += [64 * h + i, 64 * h + 32 + i]
    return np.array(p)


def prep_shared(wq, bq, wk, wv, bv, wo, bo):
    scale = float(D) ** -0.5
    perm = _perm()
    wqT = np.ascontiguousarray((wq * scale).T[:, perm])       # [e, i']
    wkT = np.ascontiguousarray(wk.T[:, perm])
    wvT = np.ascontiguousarray(wv.T)
    woT = np.ascontiguousarray(wo.T)                          # [i, j]
    sh = {}
    sh["wq"] = np.ascontiguousarray(
        wqT.reshape(NE, P, NI, P).transpose(1, 2, 0, 3)).astype(BF)
    sh["wk"] = np.ascontiguousarray(
        wkT.reshape(NE, P, NI, P).transpose(1, 2, 0, 3)).astype(BF)
    sh["wv"] = np.ascontiguousarray(
        wvT.reshape(NE, P, 2, HALF).transpose(1, 2, 0, 3)).astype(BF)
    sh["wo"] = np.ascontiguousarray(
        woT.reshape(NI, P, E).transpose(1, 0, 2)).astype(BF)
    bqp = (bq * scale)[perm]
    swap = np.arange(E) ^ 1
    sh["bqc"] = np.ascontiguousarray(bqp.reshape(NI, P).T).astype(np.float32)
    sh["bqr"] = np.ascontiguousarray(bqp[swap].reshape(NI, P).T).astype(np.float32)
    sh["bor"] = (bo + wo @ bv).reshape(1, E).astype(np.float32)
    return sh


_NC = None


def kernel(hidden_states, cos, sin, wq, bq, wk, wv, bv, wo, bo,
           cu_seqlens, max_seqlen):
    global _NC
    hidden_states = np.asarray(hidden_states, dtype=np.float32)
    cos = np.asarray(cos, dtype=np.float32)
    sin = np.asarray(sin, dtype=np.float32)
    cu = np.asarray(cu_seqlens)
    assert hidden_states.shape == (NCORES * T, E)
    assert np.array_equal(cu, np.arange(NCORES + 1, dtype=cu.dtype) * T), \
        "kernel specialized for 8 equal sequences of 1024"

    if _NC is None:
        _NC = build_nc()
    shared = prep_shared(np.asarray(wq, np.float32), np.asarray(bq, np.float32),
                         np.asarray(wk, np.float32), np.asarray(wv, np.float32),
                         np.asarray(bv, np.float32), np.asarray(wo, np.float32),
                         np.asarray(bo, np.float32))
    in_maps = []
    for s in range(NCORES):
        sl = slice(s * T, (s + 1) * T)
        in_maps.append(prep_core_inputs(hidden_states[sl], cos[sl], sin[sl],
                                        shared))
    res = run_bass_kernel_spmd(_NC, in_maps, list(range(NCORES)))
    return np.concatenate(
        [res.results[s]["y"].astype(np.float32) for s in range(NCORES)], axis=0)


if __name__ == "__main__":
    print("building program...")
    nc = build_nc()
    print("ok")

